# revision 19
# baseline (speedup 1.0000x reference)
"""Trainium2 Bass kernel for nn_MessageAggregationAttention.

Shards B=256 graphs across 8 NeuronCores (32 graphs each). Each core:
  - indirect-DMA gathers its query rows and incoming-message rows from a
    replicated (edge_attr + zero-row) table,
  - runs per-graph 4-head attention (padded LQ=128 / LK=384) with the
    softmax computed in "transposed logits" layout (keys on partitions) so
    no attention-matrix transposes are needed,
  - applies out-proj + residual + FFN,
  - indirect-DMA scatters the result rows back to the full output.

Host side only builds small int32 index tables / masks and pre-transposed
weights; all heavy data movement and compute happens on device.
"""

import math

import ml_dtypes
import numpy as np

import concourse.bass as bass
import concourse.mybir as mybir
from concourse import bacc
from concourse.bass_utils import run_bass_kernel_spmd
from concourse.masks import make_identity
from concourse.tile import TileContext

B, E, M, H, NH = 256, 16384, 65536, 128, 4
HD = H // NH               # 32
LQ, LK = 128, 384
NCORES = 8
G = B // NCORES            # 32 graphs per core
NQT = G * LQ // 128        # 32 query tiles of 128 slots
NKT = G * LK // 128        # 96 key tiles of 128 slots
XZ_ROWS = E + 128          # edge_attr + zero rows (row E.. are zeros)
MASK_VAL = -100.0          # exp(logit + MASK_VAL) == 0.0 in f32 for our logit range

f32 = mybir.dt.float32
bf16 = mybir.dt.bfloat16
i32 = mybir.dt.int32

AFT = mybir.ActivationFunctionType

# run_bass_kernel_spmd results from the last invocation (for test harness).
LAST_RESULTS = None
TRACE = False
TRACE_KW = {}


def _build_program():
    nc = bacc.Bacc("TRN2")

    xz = nc.dram_tensor("xz", [XZ_ROWS, H], bf16, kind="ExternalInput")
    xq_d = nc.dram_tensor("xq_in", [NQT * 128, H], f32, kind="ExternalInput")
    idxk_d = nc.dram_tensor("idxk", [128, NKT], i32, kind="ExternalInput")
    maskk_d = nc.dram_tensor("maskk", [128, NKT], f32, kind="ExternalInput")
    wqTz_d = nc.dram_tensor("wqTz", [H, 4 * H], f32, kind="ExternalInput")
    wkT_d = nc.dram_tensor("wkT", [H, H], bf16, kind="ExternalInput")
    wvT_d = nc.dram_tensor("wvT", [H, H], bf16, kind="ExternalInput")
    woT_d = nc.dram_tensor("woT", [H, H], bf16, kind="ExternalInput")
    w1T_d = nc.dram_tensor("w1T", [H, 2 * H], bf16, kind="ExternalInput")
    w2T_d = nc.dram_tensor("w2T", [2 * H, H], bf16, kind="ExternalInput")
    bq_d = nc.dram_tensor("bqz", [H, 4], f32, kind="ExternalInput")
    bk_d = nc.dram_tensor("bkc", [H, 1], f32, kind="ExternalInput")
    bo_d = nc.dram_tensor("boc", [H, 1], f32, kind="ExternalInput")
    b1_d = nc.dram_tensor("b1c", [H, 2], f32, kind="ExternalInput")
    b2_d = nc.dram_tensor("b2c", [H, 1], f32, kind="ExternalInput")

    out = nc.dram_tensor("out", [NQT * 128, H], f32, kind="ExternalOutput")

    with TileContext(nc) as tc:
        with (
            tc.tile_pool(name="const", bufs=1) as constp,
            tc.tile_pool(name="xtok", bufs=8) as xtokp,
            tc.tile_pool(name="xkT", bufs=8) as xkTp,
            tc.tile_pool(name="qblk", bufs=1) as qblkp,
            tc.tile_pool(name="kv", bufs=3) as kvp,
            tc.tile_pool(name="exp", bufs=6) as expp,
            tc.tile_pool(name="attn", bufs=4) as attnp,
            tc.tile_pool(name="ffn", bufs=3) as ffnp,
            tc.tile_pool(name="ps_sm", bufs=2, space="PSUM") as ps_smp,
            tc.tile_pool(name="ps_big", bufs=3, space="PSUM") as ps_bigp,
            tc.tile_pool(name="ps_acc", bufs=2, space="PSUM") as ps_accp,
            tc.tile_pool(name="ps_den", bufs=1, space="PSUM") as ps_denp,
        ):
            # ---- constants ----
            ident = constp.tile([128, 128], f32)
            make_identity(nc, ident[:])
            ones_col = constp.tile([128, 1], bf16)
            nc.vector.memset(ones_col[:], 1.0)
            ones_row = constp.tile([1, 32], f32)
            nc.vector.memset(ones_row[:], 1.0)

            def _load(shape, dram, dt=f32):
                t = constp.tile(shape, dt, tag=dram.name, name=dram.name + "_sb")
                nc.sync.dma_start(out=t[:], in_=dram[:])
                return t

            wqTz = _load([H, 4 * H], wqTz_d)
            wkT = _load([H, H], wkT_d, bf16)
            wvT = _load([H, H], wvT_d, bf16)
            woT = _load([H, H], woT_d, bf16)
            w1T = _load([H, 2 * H], w1T_d, bf16)
            w2T_a = constp.tile([128, H], bf16, tag="w2Ta")
            w2T_b = constp.tile([128, H], bf16, tag="w2Tb")
            nc.sync.dma_start(out=w2T_a[:], in_=w2T_d[0:128, :])
            nc.sync.dma_start(out=w2T_b[:], in_=w2T_d[128:256, :])
            bqz = _load([H, 4], bq_d)
            bkc = _load([H, 1], bk_d)
            boc = _load([H, 1], bo_d)
            b1c = _load([H, 2], b1_d)
            b2c = _load([H, 1], b2_d)
            maskk = _load([128, NKT], maskk_d)
            idxk = constp.tile([128, NKT], i32, tag="idxk")
            nc.sync.dma_start(out=idxk[:], in_=idxk_d[:])

            # ---- persistent per-phase activations (eight 512-col blocks) ----
            xqT_blk = [constp.tile([128, 512], f32, tag=f"xqT{b}", name=f"xqT{b}") for b in range(8)]
            qTz = constp.tile([128, 4, G * 128], bf16, tag="qTz", name="qTz")
            ar_blk = [constp.tile([128, 512], f32, tag=f"ar{b}", name=f"arb{b}") for b in range(8)]
            fin_blk = [constp.tile([128, 512], f32, tag=f"fin{b}", name=f"fin{b}") for b in range(8)]

            # ---- Q path: contiguous slab -> transpose -> xqT blocks ----
            for qt in range(NQT):
                xq_tok = xtokp.tile([128, 128], f32, tag="xq_tok")
                nc.sync.dma_start(
                    out=xq_tok[:], in_=xq_d[qt * 128 : (qt + 1) * 128, :]
                )
                ps = ps_smp.tile([128, 128], f32, tag="ps_tr")
                nc.tensor.transpose(out=ps[:], in_=xq_tok[:], identity=ident[:])
                nc.vector.tensor_copy(
                    out=xqT_blk[qt // 4][:, (qt % 4) * 128 : (qt % 4 + 1) * 128],
                    in_=ps[:],
                )

            # ---- Q projection: per head (masked weights -> zero-blocked qTz) ----
            for h in range(4):
                for blk in range(8):
                    ps = ps_bigp.tile([128, 512], f32, tag="ps_big")
                    nc.tensor.matmul(
                        out=ps[:], lhsT=wqTz[:, h * 128 : (h + 1) * 128],
                        rhs=xqT_blk[blk][:], start=True, stop=True,
                    )
                    nc.vector.tensor_scalar_add(
                        out=qTz[:, h, blk * 512 : (blk + 1) * 512],
                        in0=ps[:],
                        scalar1=bqz[:, h : h + 1],
                    )

            # ---- per graph: K/V path + attention (software pipelined) ----
            kT_g = {}
            v_g = {}

            def emit_k(g):
                kT = kvp.tile([128, LK], bf16, tag="kT", name="kT")
                v_t = [kvp.tile([128, 128], bf16, tag=f"v{t}", name=f"vt{t}")
                       for t in range(3)]
                kT_g[g] = kT
                v_g[g] = v_t
                xk_tok = xtokp.tile([128, LK], bf16, tag="xk_tok", name="xk_tok")
                for t in range(3):
                    kt = g * 3 + t
                    nc.gpsimd.indirect_dma_start(
                        out=xk_tok[:, t * 128 : (t + 1) * 128],
                        out_offset=None,
                        in_=xz[:],
                        in_offset=bass.IndirectOffsetOnAxis(
                            ap=idxk[:, kt : kt + 1], axis=0
                        ),
                    )
                xkT = xkTp.tile([128, 3, 128], bf16, tag="xkT", name="xkT")
                dmae = nc.sync
                dmae.dma_start_transpose(
                    out=xkT[:], in_=xk_tok[:].rearrange("p (t f) -> p t f", t=3)
                )
                psk = ps_bigp.tile([128, 512], f32, tag="ps_big", name="psk")
                nc.tensor.matmul(
                    out=psk[:, 0:LK], lhsT=wkT[:],
                    rhs=xkT[:].rearrange("f t p -> f (t p)"),
                    start=True, stop=True,
                )
                nc.vector.tensor_scalar_add(
                    out=kT[:], in0=psk[:, 0:LK], scalar1=bkc[:, 0:1],
                )
                for t in range(3):
                    psv = ps_smp.tile([128, 128], f32, tag="ps_tr", name="psv")
                    nc.tensor.matmul(
                        out=psv[:], lhsT=xkT[:, t, :], rhs=wvT[:],
                        start=True, stop=True,
                    )
                    nc.vector.tensor_copy(out=v_t[t][:], in_=psv[:])

            def emit_attn(g):
                kT = kT_g.pop(g)
                v_t = v_g.pop(g)
                ctx_ps = ps_accp.tile([128, 128], f32, tag="ctx", name="ctx")
                den_ps = ps_denp.tile([1, 512], f32, tag="den", name="den")
                for t in range(3):
                    lg_ps = ps_bigp.tile([128, 512], f32, tag="ps_big", name="lg")
                    nc.tensor.matmul(
                        out=lg_ps[:],
                        lhsT=kT[:, t * 128 : (t + 1) * 128],
                        rhs=qTz[:, :, g * 128 : (g + 1) * 128],
                        start=True, stop=True,
                    )
                    ex = expp.tile([128, 512], bf16, tag="exp", name="ex")
                    kt = g * 3 + t
                    nc.scalar.activation(
                        out=ex[:], in_=lg_ps[:], func=AFT.Exp,
                        bias=maskk[:, kt : kt + 1],
                    )
                    nc.tensor.matmul(
                        out=den_ps[:], lhsT=ones_col[:], rhs=ex[:],
                        start=(t == 0), stop=(t == 2), skip_group_check=True,
                    )
                    for h in range(4):
                        nc.tensor.matmul(
                            out=ctx_ps[32 * h : 32 * (h + 1), :],
                            lhsT=v_t[t][:, 32 * h : 32 * (h + 1)],
                            rhs=ex[:, h * 128 : (h + 1) * 128],
                            start=(t == 0), stop=(t == 2), skip_group_check=True,
                            tile_position=(0, 32 * h),
                        )
                rden = attnp.tile([1, 512], f32, tag="rden", name="rden")
                nc.vector.reciprocal_approx_fast(out=rden[:], in_=den_ps[:])
                bc_ps = ps_smp.tile([128, 128], f32, tag="ps_tr", name="bc")
                for h in range(4):
                    nc.tensor.matmul(
                        out=bc_ps[32 * h : 32 * (h + 1), :],
                        lhsT=ones_row[:],
                        rhs=rden[:, h * 128 : (h + 1) * 128],
                        start=True, stop=True,
                        tile_position=(0, 32 * h),
                    )
                bc_sb = attnp.tile([128, 128], f32, tag="bc_sb", name="bc_sb")
                nc.vector.tensor_copy(out=bc_sb[:], in_=bc_ps[:])
                ctxn = attnp.tile([128, 128], bf16, tag="ctxn", name="ctxn")
                nc.vector.tensor_mul(out=ctxn[:], in0=ctx_ps[:], in1=bc_sb[:])
                po = ps_smp.tile([128, 128], f32, tag="ps_tr", name="po")
                nc.tensor.matmul(
                    out=po[:], lhsT=woT[:], rhs=ctxn[:], start=True, stop=True
                )
                ao = attnp.tile([128, 128], f32, tag="ao", name="ao")
                nc.vector.tensor_scalar_add(
                    out=ao[:], in0=po[:], scalar1=boc[:, 0:1],
                )
                qc = (g % 4) * 128
                nc.vector.tensor_add(
                    out=ar_blk[g // 4][:, qc : qc + 128],
                    in0=ao[:],
                    in1=xqT_blk[g // 4][:, qc : qc + 128],
                )

            LAG = 2
            for i in range(G + LAG):
                if i < G:
                    emit_k(i)
                if i >= LAG:
                    emit_attn(i - LAG)

            # ---- FFN (batched over 512-col blocks) ----
            arbf_blk = [constp.tile([128, 512], bf16, tag=f"arbf{b}", name=f"arbf{b}")
                        for b in range(8)]
            for blk in range(8):
                nc.vector.tensor_copy(out=arbf_blk[blk][:], in_=ar_blk[blk][:])
                pa = ps_bigp.tile([128, 512], f32, tag="ps_big")
                nc.tensor.matmul(
                    out=pa[:], lhsT=w1T[:, 0:128], rhs=arbf_blk[blk][:],
                    start=True, stop=True,
                )
                ra = ffnp.tile([128, 512], bf16, tag="ra")
                nc.scalar.activation(
                    out=ra[:], in_=pa[:], func=AFT.Relu, bias=b1c[:, 0:1]
                )
                pb = ps_bigp.tile([128, 512], f32, tag="ps_big")
                nc.tensor.matmul(
                    out=pb[:], lhsT=w1T[:, 128:256], rhs=arbf_blk[blk][:],
                    start=True, stop=True,
                )
                rb = ffnp.tile([128, 512], bf16, tag="rb")
                nc.scalar.activation(
                    out=rb[:], in_=pb[:], func=AFT.Relu, bias=b1c[:, 1:2]
                )
                p2 = ps_bigp.tile([128, 512], f32, tag="ps_big")
                nc.tensor.matmul(
                    out=p2[:], lhsT=w2T_a[:], rhs=ra[:], start=True, stop=False,
                    skip_group_check=True,
                )
                nc.tensor.matmul(
                    out=p2[:], lhsT=w2T_b[:], rhs=rb[:], start=False, stop=True,
                    skip_group_check=True,
                )
                f2 = ffnp.tile([128, 512], f32, tag="f2")
                nc.scalar.activation(
                    out=f2[:], in_=p2[:], func=AFT.Identity, bias=b2c[:, 0:1]
                )
                nc.vector.tensor_add(
                    out=fin_blk[blk][:], in0=f2[:], in1=ar_blk[blk][:]
                )

            # ---- transpose back + scatter ----
            for qt in range(NQT):
                ps = ps_smp.tile([128, 128], f32, tag="ps_tr")
                nc.tensor.transpose(
                    out=ps[:],
                    in_=fin_blk[qt // 4][:, (qt % 4) * 128 : (qt % 4 + 1) * 128],
                    identity=ident[:],
                )
                ftok = xtokp.tile([128, 128], f32, tag="ftok")
                nc.vector.tensor_copy(out=ftok[:], in_=ps[:])
                nc.sync.dma_start(
                    out=out[qt * 128 : (qt + 1) * 128, :], in_=ftok[:]
                )
    nc.finalize()
    return nc


_NC_CACHE = None


def kernel(edge_index, edge_attr, incoming_edges_list, incoming_edges_batch,
           edge_batch, in_proj_w, in_proj_b, out_proj_w, out_proj_b,
           w1, b1, w2, b2):
    global _NC_CACHE, LAST_RESULTS

    edge_attr = np.asarray(edge_attr, np.float32)
    edge_batch = np.asarray(edge_batch, np.int64)
    incoming_edges_list = np.asarray(incoming_edges_list, np.int64)
    incoming_edges_batch = np.asarray(incoming_edges_batch, np.int64)

    # ---- host prep: index tables, masks, weights ----
    xz32 = np.zeros((XZ_ROWS, H), np.float32)
    xz32[:E] = edge_attr

    cnt_q = np.bincount(edge_batch, minlength=B)
    st_q = np.zeros(B + 1, np.int64)
    np.cumsum(cnt_q, out=st_q[1:])
    cnt_k = np.bincount(incoming_edges_batch, minlength=B)
    st_k = np.zeros(B + 1, np.int64)
    np.cumsum(cnt_k, out=st_k[1:])
    assert cnt_q.max() <= LQ and cnt_k.max() <= LK and cnt_k.min() >= 1

    # Q slabs: [B, LQ] contiguous rows starting at each graph's first edge
    pos_q = np.arange(LQ)[None, :]
    slab_rows = (st_q[:B, None] + pos_q)           # [B, LQ], < XZ_ROWS always
    pos_k = np.arange(LK)[None, :]
    gath = np.full((B, LK), E, np.int64)
    valid = pos_k < cnt_k[:, None]
    flat_idx = (st_k[:B, None] + np.minimum(pos_k, cnt_k[:, None] - 1))
    gath[valid] = incoming_edges_list[flat_idx[valid]]
    idxk_full = gath.astype(np.int32)
    maskk_full = np.where(valid, 0.0, MASK_VAL).astype(np.float32)

    xzbf = xz32.astype(ml_dtypes.bfloat16)
    s = 1.0 / math.sqrt(HD)
    wq, wk, wv = in_proj_w[:H], in_proj_w[H:2 * H], in_proj_w[2 * H:]
    bq, bk, bv = in_proj_b[:H], in_proj_b[H:2 * H], in_proj_b[2 * H:]
    wqT = np.ascontiguousarray((wq * s).T, np.float32)
    wqTz = np.zeros((H, 4 * H), np.float32)
    bqz = np.zeros((H, 4), np.float32)
    for h in range(4):
        wqTz[:, h * H + 32 * h : h * H + 32 * (h + 1)] = \
            wqT[:, 32 * h : 32 * (h + 1)]
        bqz[32 * h : 32 * (h + 1), h] = (bq * s)[32 * h : 32 * (h + 1)]
    bft = ml_dtypes.bfloat16
    wkT = np.ascontiguousarray(wk.T.astype(bft))
    wvT = np.ascontiguousarray(wv.T.astype(bft))
    woT = np.ascontiguousarray(out_proj_w.T.astype(bft))
    w1T = np.ascontiguousarray(w1.T.astype(bft))          # [H, 2H]
    w2T = np.ascontiguousarray(w2.T.astype(bft))          # [2H, H]
    bkc = np.ascontiguousarray(bk[:, None], np.float32)
    boc = np.ascontiguousarray(
        (out_proj_b + out_proj_w @ bv)[:, None], np.float32
    )
    b1c = np.ascontiguousarray(b1.reshape(2, H).T, np.float32)
    b2c = np.ascontiguousarray(b2[:, None], np.float32)

    shared = dict(xz=xzbf, wqTz=wqTz, wkT=wkT, wvT=wvT, woT=woT, w1T=w1T,
                  w2T=w2T, bqz=bqz, bkc=bkc, boc=boc, b1c=b1c, b2c=b2c)
    in_maps = []
    for c in range(NCORES):
        gs = slice(c * G, (c + 1) * G)
        xq_c = np.ascontiguousarray(
            xz32[slab_rows[gs].reshape(-1)])         # [4096, 128]
        # [G, L] -> [128, n_tiles]: tile j of 128 slots -> column j
        idxk_c = np.ascontiguousarray(
            idxk_full[gs].reshape(NKT, 128).T)
        maskk_c = np.ascontiguousarray(
            maskk_full[gs].reshape(NKT, 128).T)
        in_maps.append(dict(shared, xq_in=xq_c, idxk=idxk_c, maskk=maskk_c))

    if _NC_CACHE is None:
        _NC_CACHE = _build_program()
    res = run_bass_kernel_spmd(
        _NC_CACHE, in_maps, core_ids=list(range(NCORES)),
        trace=TRACE, **TRACE_KW,
    )
    LAST_RESULTS = res

    # compact: edge e lives at dense slot (g_local*LQ + pos) of its core
    eb = edge_batch
    g_local = (eb % G).astype(np.int64)
    pos = np.arange(E) - st_q[eb]
    slot = g_local * LQ + pos
    out_full = np.empty((E, H), np.float32)
    for c in range(NCORES):
        sel = (eb // G) == c
        out_full[sel] = res.results[c]["out"][slot[sel]]
    return out_full


# revision 20
# speedup vs baseline: 1.0120x; 1.0120x over previous
"""Trainium2 Bass kernel for nn_MessageAggregationAttention.

Shards B=256 graphs across 8 NeuronCores (32 graphs each). Each core:
  - indirect-DMA gathers its query rows and incoming-message rows from a
    replicated (edge_attr + zero-row) table,
  - runs per-graph 4-head attention (padded LQ=128 / LK=384) with the
    softmax computed in "transposed logits" layout (keys on partitions) so
    no attention-matrix transposes are needed,
  - applies out-proj + residual + FFN,
  - indirect-DMA scatters the result rows back to the full output.

Host side only builds small int32 index tables / masks and pre-transposed
weights; all heavy data movement and compute happens on device.
"""

import math

import ml_dtypes
import numpy as np

import concourse.bass as bass
import concourse.mybir as mybir
from concourse import bacc
from concourse.bass_utils import run_bass_kernel_spmd
from concourse.masks import make_identity
from concourse.tile import TileContext

B, E, M, H, NH = 256, 16384, 65536, 128, 4
HD = H // NH               # 32
LQ, LK = 128, 384
NCORES = 8
G = B // NCORES            # 32 graphs per core
NQT = G * LQ // 128        # 32 query tiles of 128 slots
NKT = G * LK // 128        # 96 key tiles of 128 slots
XZ_ROWS = E + 128          # edge_attr + zero rows (row E.. are zeros)
MASK_VAL = -100.0          # exp(logit + MASK_VAL) == 0.0 in f32 for our logit range

f32 = mybir.dt.float32
bf16 = mybir.dt.bfloat16
i32 = mybir.dt.int32

AFT = mybir.ActivationFunctionType

# run_bass_kernel_spmd results from the last invocation (for test harness).
LAST_RESULTS = None
TRACE = False
TRACE_KW = {}


def _build_program():
    nc = bacc.Bacc("TRN2")

    xz = nc.dram_tensor("xz", [XZ_ROWS, H], bf16, kind="ExternalInput")
    xq_d = nc.dram_tensor("xq_in", [NQT * 128, H], f32, kind="ExternalInput")
    idxk_d = nc.dram_tensor("idxk", [128, NKT], i32, kind="ExternalInput")
    maskk_d = nc.dram_tensor("maskk", [128, NKT], f32, kind="ExternalInput")
    wqTz_d = nc.dram_tensor("wqTz", [H, 4 * H], f32, kind="ExternalInput")
    wkT_d = nc.dram_tensor("wkT", [H, H], bf16, kind="ExternalInput")
    wvT_d = nc.dram_tensor("wvT", [H, H], bf16, kind="ExternalInput")
    woT_d = nc.dram_tensor("woT", [H, H], bf16, kind="ExternalInput")
    w1T_d = nc.dram_tensor("w1T", [H, 2 * H], bf16, kind="ExternalInput")
    w2T_d = nc.dram_tensor("w2T", [2 * H, H], bf16, kind="ExternalInput")
    bq_d = nc.dram_tensor("bqz", [H, 4], f32, kind="ExternalInput")
    bk_d = nc.dram_tensor("bkc", [H, 1], f32, kind="ExternalInput")
    bo_d = nc.dram_tensor("boc", [H, 1], f32, kind="ExternalInput")
    b1_d = nc.dram_tensor("b1c", [H, 2], f32, kind="ExternalInput")
    b2_d = nc.dram_tensor("b2c", [H, 1], f32, kind="ExternalInput")

    out = nc.dram_tensor("out", [NQT * 128, H], f32, kind="ExternalOutput")

    with TileContext(nc) as tc:
        with (
            tc.tile_pool(name="const", bufs=1) as constp,
            tc.tile_pool(name="xtok", bufs=8) as xtokp,
            tc.tile_pool(name="xkT", bufs=8) as xkTp,
            tc.tile_pool(name="qblk", bufs=1) as qblkp,
            tc.tile_pool(name="kv", bufs=4) as kvp,
            tc.tile_pool(name="exp", bufs=9) as expp,
            tc.tile_pool(name="attn", bufs=6) as attnp,
            tc.tile_pool(name="ffn", bufs=3) as ffnp,
            tc.tile_pool(name="ps_sm", bufs=2, space="PSUM") as ps_smp,
            tc.tile_pool(name="ps_big", bufs=2, space="PSUM") as ps_bigp,
            tc.tile_pool(name="ps_acc", bufs=2, space="PSUM") as ps_accp,
            tc.tile_pool(name="ps_den", bufs=2, space="PSUM") as ps_denp,
        ):
            # ---- constants ----
            ident = constp.tile([128, 128], f32)
            make_identity(nc, ident[:])
            ones_col = constp.tile([128, 1], bf16)
            nc.vector.memset(ones_col[:], 1.0)
            ones_row = constp.tile([1, 32], f32)
            nc.vector.memset(ones_row[:], 1.0)

            def _load(shape, dram, dt=f32):
                t = constp.tile(shape, dt, tag=dram.name, name=dram.name + "_sb")
                nc.sync.dma_start(out=t[:], in_=dram[:])
                return t

            wqTz = _load([H, 4 * H], wqTz_d)
            wkT = _load([H, H], wkT_d, bf16)
            wvT = _load([H, H], wvT_d, bf16)
            woT = _load([H, H], woT_d, bf16)
            w1T = _load([H, 2 * H], w1T_d, bf16)
            w2T_a = constp.tile([128, H], bf16, tag="w2Ta")
            w2T_b = constp.tile([128, H], bf16, tag="w2Tb")
            nc.sync.dma_start(out=w2T_a[:], in_=w2T_d[0:128, :])
            nc.sync.dma_start(out=w2T_b[:], in_=w2T_d[128:256, :])
            bqz = _load([H, 4], bq_d)
            bkc = _load([H, 1], bk_d)
            boc = _load([H, 1], bo_d)
            b1c = _load([H, 2], b1_d)
            b2c = _load([H, 1], b2_d)
            maskk = _load([128, NKT], maskk_d)
            idxk = constp.tile([128, NKT], i32, tag="idxk")
            nc.sync.dma_start(out=idxk[:], in_=idxk_d[:])

            # ---- persistent per-phase activations (eight 512-col blocks) ----
            xqT_blk = [constp.tile([128, 512], f32, tag=f"xqT{b}", name=f"xqT{b}") for b in range(8)]
            qTz = constp.tile([128, 4, G * 128], bf16, tag="qTz", name="qTz")
            ar_blk = [constp.tile([128, 512], f32, tag=f"ar{b}", name=f"arb{b}") for b in range(8)]
            fin_blk = [constp.tile([128, 512], f32, tag=f"fin{b}", name=f"fin{b}") for b in range(8)]

            # ---- Q path: contiguous slab -> transpose -> xqT blocks ----
            for qt in range(NQT):
                xq_tok = xtokp.tile([128, 128], f32, tag="xq_tok")
                nc.sync.dma_start(
                    out=xq_tok[:], in_=xq_d[qt * 128 : (qt + 1) * 128, :]
                )
                ps = ps_smp.tile([128, 128], f32, tag="ps_tr")
                nc.tensor.transpose(out=ps[:], in_=xq_tok[:], identity=ident[:])
                nc.vector.tensor_copy(
                    out=xqT_blk[qt // 4][:, (qt % 4) * 128 : (qt % 4 + 1) * 128],
                    in_=ps[:],
                )

            # ---- Q projection: per head (masked weights -> zero-blocked qTz) ----
            for h in range(4):
                for blk in range(8):
                    ps = ps_bigp.tile([128, 512], f32, tag="ps_big")
                    nc.tensor.matmul(
                        out=ps[:], lhsT=wqTz[:, h * 128 : (h + 1) * 128],
                        rhs=xqT_blk[blk][:], start=True, stop=True,
                    )
                    nc.vector.tensor_scalar_add(
                        out=qTz[:, h, blk * 512 : (blk + 1) * 512],
                        in0=ps[:],
                        scalar1=bqz[:, h : h + 1],
                    )

            # ---- per graph: K/V path + attention (software pipelined) ----
            kT_g = {}
            v_g = {}

            def emit_k(g):
                kT = kvp.tile([128, LK], bf16, tag="kT", name="kT")
                v_t = [kvp.tile([128, 128], bf16, tag=f"v{t}", name=f"vt{t}")
                       for t in range(3)]
                kT_g[g] = kT
                v_g[g] = v_t
                xk_tok = xtokp.tile([128, LK], bf16, tag="xk_tok", name="xk_tok")
                for t in range(3):
                    kt = g * 3 + t
                    nc.gpsimd.indirect_dma_start(
                        out=xk_tok[:, t * 128 : (t + 1) * 128],
                        out_offset=None,
                        in_=xz[:],
                        in_offset=bass.IndirectOffsetOnAxis(
                            ap=idxk[:, kt : kt + 1], axis=0
                        ),
                    )
                xkT = xkTp.tile([128, 3, 128], bf16, tag="xkT", name="xkT")
                dmae = nc.sync
                dmae.dma_start_transpose(
                    out=xkT[:], in_=xk_tok[:].rearrange("p (t f) -> p t f", t=3)
                )
                psk = ps_bigp.tile([128, 512], f32, tag="ps_big", name="psk")
                nc.tensor.matmul(
                    out=psk[:, 0:LK], lhsT=wkT[:],
                    rhs=xkT[:].rearrange("f t p -> f (t p)"),
                    start=True, stop=True,
                )
                nc.vector.tensor_scalar_add(
                    out=kT[:], in0=psk[:, 0:LK], scalar1=bkc[:, 0:1],
                )
                for t in range(3):
                    psv = ps_smp.tile([128, 128], f32, tag="ps_tr", name="psv")
                    nc.tensor.matmul(
                        out=psv[:], lhsT=xkT[:, t, :], rhs=wvT[:],
                        start=True, stop=True,
                    )
                    nc.vector.tensor_copy(out=v_t[t][:], in_=psv[:])

            def emit_attn(g):
                kT = kT_g.pop(g)
                v_t = v_g.pop(g)
                ctx_ps = ps_accp.tile([128, 128], f32, tag="ctx", name="ctx")
                den_ps = ps_denp.tile([1, 512], f32, tag="den", name="den")
                for t in range(3):
                    lg_ps = ps_bigp.tile([128, 512], f32, tag="ps_big", name="lg")
                    nc.tensor.matmul(
                        out=lg_ps[:],
                        lhsT=kT[:, t * 128 : (t + 1) * 128],
                        rhs=qTz[:, :, g * 128 : (g + 1) * 128],
                        start=True, stop=True,
                    )
                    ex = expp.tile([128, 512], bf16, tag="exp", name="ex")
                    kt = g * 3 + t
                    nc.scalar.activation(
                        out=ex[:], in_=lg_ps[:], func=AFT.Exp,
                        bias=maskk[:, kt : kt + 1],
                    )
                    nc.tensor.matmul(
                        out=den_ps[:], lhsT=ones_col[:], rhs=ex[:],
                        start=(t == 0), stop=(t == 2), skip_group_check=True,
                    )
                    for h in range(4):
                        nc.tensor.matmul(
                            out=ctx_ps[32 * h : 32 * (h + 1), :],
                            lhsT=v_t[t][:, 32 * h : 32 * (h + 1)],
                            rhs=ex[:, h * 128 : (h + 1) * 128],
                            start=(t == 0), stop=(t == 2), skip_group_check=True,
                            tile_position=(0, 32 * h),
                        )
                rden = attnp.tile([1, 512], f32, tag="rden", name="rden")
                nc.vector.reciprocal_approx_fast(out=rden[:], in_=den_ps[:])
                bc_ps = ps_smp.tile([128, 128], f32, tag="ps_tr", name="bc")
                for h in range(4):
                    nc.tensor.matmul(
                        out=bc_ps[32 * h : 32 * (h + 1), :],
                        lhsT=ones_row[:],
                        rhs=rden[:, h * 128 : (h + 1) * 128],
                        start=True, stop=True,
                        tile_position=(0, 32 * h),
                    )
                bc_sb = attnp.tile([128, 128], f32, tag="bc_sb", name="bc_sb")
                nc.vector.tensor_copy(out=bc_sb[:], in_=bc_ps[:])
                ctxn = attnp.tile([128, 128], bf16, tag="ctxn", name="ctxn")
                nc.vector.tensor_mul(out=ctxn[:], in0=ctx_ps[:], in1=bc_sb[:])
                po = ps_smp.tile([128, 128], f32, tag="ps_tr", name="po")
                nc.tensor.matmul(
                    out=po[:], lhsT=woT[:], rhs=ctxn[:], start=True, stop=True
                )
                ao = attnp.tile([128, 128], f32, tag="ao", name="ao")
                nc.vector.tensor_scalar_add(
                    out=ao[:], in0=po[:], scalar1=boc[:, 0:1],
                )
                qc = (g % 4) * 128
                nc.vector.tensor_add(
                    out=ar_blk[g // 4][:, qc : qc + 128],
                    in0=ao[:],
                    in1=xqT_blk[g // 4][:, qc : qc + 128],
                )

            LAG = 3
            for i in range(G + LAG):
                if i < G:
                    emit_k(i)
                if i >= LAG:
                    emit_attn(i - LAG)

            # ---- FFN (batched over 512-col blocks) ----
            arbf_blk = [constp.tile([128, 512], bf16, tag=f"arbf{b}", name=f"arbf{b}")
                        for b in range(8)]
            for blk in range(8):
                nc.vector.tensor_copy(out=arbf_blk[blk][:], in_=ar_blk[blk][:])
                pa = ps_bigp.tile([128, 512], f32, tag="ps_big")
                nc.tensor.matmul(
                    out=pa[:], lhsT=w1T[:, 0:128], rhs=arbf_blk[blk][:],
                    start=True, stop=True,
                )
                ra = ffnp.tile([128, 512], bf16, tag="ra")
                nc.scalar.activation(
                    out=ra[:], in_=pa[:], func=AFT.Relu, bias=b1c[:, 0:1]
                )
                pb = ps_bigp.tile([128, 512], f32, tag="ps_big")
                nc.tensor.matmul(
                    out=pb[:], lhsT=w1T[:, 128:256], rhs=arbf_blk[blk][:],
                    start=True, stop=True,
                )
                rb = ffnp.tile([128, 512], bf16, tag="rb")
                nc.scalar.activation(
                    out=rb[:], in_=pb[:], func=AFT.Relu, bias=b1c[:, 1:2]
                )
                p2 = ps_bigp.tile([128, 512], f32, tag="ps_big")
                nc.tensor.matmul(
                    out=p2[:], lhsT=w2T_a[:], rhs=ra[:], start=True, stop=False,
                    skip_group_check=True,
                )
                nc.tensor.matmul(
                    out=p2[:], lhsT=w2T_b[:], rhs=rb[:], start=False, stop=True,
                    skip_group_check=True,
                )
                f2 = ffnp.tile([128, 512], f32, tag="f2")
                nc.scalar.activation(
                    out=f2[:], in_=p2[:], func=AFT.Identity, bias=b2c[:, 0:1]
                )
                nc.vector.tensor_add(
                    out=fin_blk[blk][:], in0=f2[:], in1=ar_blk[blk][:]
                )

            # ---- transpose back + scatter ----
            for qt in range(NQT):
                ps = ps_smp.tile([128, 128], f32, tag="ps_tr")
                nc.tensor.transpose(
                    out=ps[:],
                    in_=fin_blk[qt // 4][:, (qt % 4) * 128 : (qt % 4 + 1) * 128],
                    identity=ident[:],
                )
                ftok = xtokp.tile([128, 128], f32, tag="ftok")
                nc.vector.tensor_copy(out=ftok[:], in_=ps[:])
                nc.sync.dma_start(
                    out=out[qt * 128 : (qt + 1) * 128, :], in_=ftok[:]
                )
    nc.finalize()
    return nc


_NC_CACHE = None


def kernel(edge_index, edge_attr, incoming_edges_list, incoming_edges_batch,
           edge_batch, in_proj_w, in_proj_b, out_proj_w, out_proj_b,
           w1, b1, w2, b2):
    global _NC_CACHE, LAST_RESULTS

    edge_attr = np.asarray(edge_attr, np.float32)
    edge_batch = np.asarray(edge_batch, np.int64)
    incoming_edges_list = np.asarray(incoming_edges_list, np.int64)
    incoming_edges_batch = np.asarray(incoming_edges_batch, np.int64)

    # ---- host prep: index tables, masks, weights ----
    xz32 = np.zeros((XZ_ROWS, H), np.float32)
    xz32[:E] = edge_attr

    cnt_q = np.bincount(edge_batch, minlength=B)
    st_q = np.zeros(B + 1, np.int64)
    np.cumsum(cnt_q, out=st_q[1:])
    cnt_k = np.bincount(incoming_edges_batch, minlength=B)
    st_k = np.zeros(B + 1, np.int64)
    np.cumsum(cnt_k, out=st_k[1:])
    assert cnt_q.max() <= LQ and cnt_k.max() <= LK and cnt_k.min() >= 1

    # Q slabs: [B, LQ] contiguous rows starting at each graph's first edge
    pos_q = np.arange(LQ)[None, :]
    slab_rows = (st_q[:B, None] + pos_q)           # [B, LQ], < XZ_ROWS always
    pos_k = np.arange(LK)[None, :]
    gath = np.full((B, LK), E, np.int64)
    valid = pos_k < cnt_k[:, None]
    flat_idx = (st_k[:B, None] + np.minimum(pos_k, cnt_k[:, None] - 1))
    gath[valid] = incoming_edges_list[flat_idx[valid]]
    idxk_full = gath.astype(np.int32)
    maskk_full = np.where(valid, 0.0, MASK_VAL).astype(np.float32)

    xzbf = xz32.astype(ml_dtypes.bfloat16)
    s = 1.0 / math.sqrt(HD)
    wq, wk, wv = in_proj_w[:H], in_proj_w[H:2 * H], in_proj_w[2 * H:]
    bq, bk, bv = in_proj_b[:H], in_proj_b[H:2 * H], in_proj_b[2 * H:]
    wqT = np.ascontiguousarray((wq * s).T, np.float32)
    wqTz = np.zeros((H, 4 * H), np.float32)
    bqz = np.zeros((H, 4), np.float32)
    for h in range(4):
        wqTz[:, h * H + 32 * h : h * H + 32 * (h + 1)] = \
            wqT[:, 32 * h : 32 * (h + 1)]
        bqz[32 * h : 32 * (h + 1), h] = (bq * s)[32 * h : 32 * (h + 1)]
    bft = ml_dtypes.bfloat16
    wkT = np.ascontiguousarray(wk.T.astype(bft))
    wvT = np.ascontiguousarray(wv.T.astype(bft))
    woT = np.ascontiguousarray(out_proj_w.T.astype(bft))
    w1T = np.ascontiguousarray(w1.T.astype(bft))          # [H, 2H]
    w2T = np.ascontiguousarray(w2.T.astype(bft))          # [2H, H]
    bkc = np.ascontiguousarray(bk[:, None], np.float32)
    boc = np.ascontiguousarray(
        (out_proj_b + out_proj_w @ bv)[:, None], np.float32
    )
    b1c = np.ascontiguousarray(b1.reshape(2, H).T, np.float32)
    b2c = np.ascontiguousarray(b2[:, None], np.float32)

    shared = dict(xz=xzbf, wqTz=wqTz, wkT=wkT, wvT=wvT, woT=woT, w1T=w1T,
                  w2T=w2T, bqz=bqz, bkc=bkc, boc=boc, b1c=b1c, b2c=b2c)
    in_maps = []
    for c in range(NCORES):
        gs = slice(c * G, (c + 1) * G)
        xq_c = np.ascontiguousarray(
            xz32[slab_rows[gs].reshape(-1)])         # [4096, 128]
        # [G, L] -> [128, n_tiles]: tile j of 128 slots -> column j
        idxk_c = np.ascontiguousarray(
            idxk_full[gs].reshape(NKT, 128).T)
        maskk_c = np.ascontiguousarray(
            maskk_full[gs].reshape(NKT, 128).T)
        in_maps.append(dict(shared, xq_in=xq_c, idxk=idxk_c, maskk=maskk_c))

    if _NC_CACHE is None:
        _NC_CACHE = _build_program()
    res = run_bass_kernel_spmd(
        _NC_CACHE, in_maps, core_ids=list(range(NCORES)),
        trace=TRACE, **TRACE_KW,
    )
    LAST_RESULTS = res

    # compact: edge e lives at dense slot (g_local*LQ + pos) of its core
    eb = edge_batch
    g_local = (eb % G).astype(np.int64)
    pos = np.arange(E) - st_q[eb]
    slot = g_local * LQ + pos
    out_full = np.empty((E, H), np.float32)
    for c in range(NCORES):
        sel = (eb // G) == c
        out_full[sel] = res.results[c]["out"][slot[sel]]
    return out_full
